# revision 1
# baseline (speedup 1.0000x reference)
"""Banded-attention (AttentionWindow) TRN2 kernel, data-parallel over batch on 8 NeuronCores.

Reference computation (per batch b):
  Q = x @ W;  scores = Q @ x^T;  scores[|i-j| > 64] = -1e9
  probs = softmax(scores, axis=-1);  out = x + relu(probs @ x)

Kernel strategy:
  - One batch per core (batch=8, n_cores=8), W replicated. No collectives.
  - The |i-j| <= 64 band means each 128-query tile only attends to a
    384-wide, 128-aligned key window: scores/softmax/PV are computed on
    [128, 384] tiles instead of [2048, 2048].
  - All matmuls run as float32r (full-rate fp32 PE mode, fp32 PSUM accum):
    Q^T chunks on PE, banded scores, and probs @ x. The softmax is fp32
    (DVE mask+max, ACT exp with per-partition bias and sum accumulation);
    normalization is folded into the final ReLU's per-partition scale.
  - probs are transposed on the PE (identity matmul) for the PV contraction.

Inputs: repr [8, 2048, 1024] f32, W [1024, 1024] f32.
Output: [8, 2048, 1024] f32.
"""
from contextlib import ExitStack

import numpy as np

SEQ, HID = 2048, 1024
W2 = 64                  # window half-width
QTL = 128                # queries per softmax tile
KW = 384                 # key window per q-tile (128-aligned superset of the band)
NQ = SEQ // QTL          # 16
GQ = 512                 # queries per Q^T-projection group
NG = SEQ // GQ           # 8
ND = HID // 128          # 8 contraction chunks
NEG = -1e9


def _legalize_waits(nc):
    """This walrus build accepts 1 sync wait per instruction (2 on
    EventSemaphore). Hoist excess waits onto EventSemaphore prefixes on the
    same engine."""
    from concourse import mybir

    n = 0
    for func in nc.m.functions:
        for blk in func.blocks:
            out = []
            changed = False
            for inst in list(blk.instructions):
                si = inst.sync_info
                cap = 2 if isinstance(inst, mybir.InstEventSemaphore) else 1
                if si is not None and len(si.on_wait) > cap:
                    waits = list(si.on_wait)
                    for i in range(cap, len(waits), 2):
                        ev = mybir.InstEventSemaphore(
                            name=f"{inst.name}_waitfix{i}",
                            engine=inst.engine,
                            ins=[],
                            outs=[],
                            sync_info=mybir.SyncInfo(on_wait=waits[i:i + 2],
                                                     on_update=[]),
                        )
                        out.append(ev)
                        n += 1
                    inst.sync_info = mybir.SyncInfo(on_wait=waits[:cap],
                                                    on_update=list(si.on_update))
                    changed = True
                out.append(inst)
            if changed:
                blk.instructions = out
    return n


def _build(nc):
    import concourse.tile as tile
    from concourse import masks, mybir

    F32 = mybir.dt.float32
    F32R = mybir.dt.float32r
    AF = mybir.ActivationFunctionType
    ALU = mybir.AluOpType
    X = mybir.AxisListType.X

    w = nc.dram_tensor("w", [HID, HID], F32R, kind="ExternalInput").ap()
    xt = nc.dram_tensor("xt", [HID, SEQ], F32R, kind="ExternalInput").ap()
    xn = nc.dram_tensor("xn", [SEQ, HID], F32R, kind="ExternalInput").ap()
    out = nc.dram_tensor("out", [SEQ, HID], F32, kind="ExternalOutput").ap()

    with tile.TileContext(nc) as tc, ExitStack() as ctx:
        pool = ctx.enter_context(tc.tile_pool(name="sb", bufs=1))
        ps = ctx.enter_context(tc.tile_pool(name="ps", bufs=1, space="PSUM"))

        # resident inputs
        wt = [pool.tile([128, HID], F32R, tag=f"w{d}", name=f"w{d}") for d in range(ND)]
        xtt = [pool.tile([128, SEQ], F32R, tag=f"xt{d}", name=f"xt{d}") for d in range(ND)]
        xnt = [pool.tile([128, HID], F32R, tag=f"xn{k}", name=f"xn{k}") for k in range(NQ)]
        for d in range(ND):
            nc.sync.dma_start(wt[d][:, 0:512], w[128 * d:128 * (d + 1), 0:512])
            nc.sync.dma_start(xtt[d][:, 0:GQ], xt[128 * d:128 * (d + 1), 0:GQ])
        for d in range(ND):
            nc.sync.dma_start(wt[d][:, 512:HID], w[128 * d:128 * (d + 1), 512:HID])
        for g in range(1, NG):
            for d in range(ND):
                nc.sync.dma_start(xtt[d][:, GQ * g:GQ * (g + 1)],
                                  xt[128 * d:128 * (d + 1), GQ * g:GQ * (g + 1)])
        for k in range(NQ):
            nc.sync.dma_start(xnt[k][:], xn[128 * k:128 * (k + 1), :])

        # identity + banded masks (keep iff |r + off - c| <= W2)
        idn = pool.tile([128, 128], F32, tag="idn", name="idn")
        masks.make_identity(nc, idn[:])
        KX = 256  # exact band window for the scores matmul
        mask_by_off = {}
        for off in (0, 64, 128):
            m = pool.tile([128, KX], F32, tag=f"mask{off}", name=f"mask{off}")
            nc.gpsimd.memset(m[:], 0.0)
            nc.gpsimd.affine_select(out=m[:], in_=m[:], compare_op=ALU.is_ge,
                                    fill=NEG, base=W2 - off, channel_multiplier=-1,
                                    pattern=[[1, KX]])
            nc.gpsimd.affine_select(out=m[:], in_=m[:], compare_op=ALU.is_ge,
                                    fill=NEG, base=W2 + off, channel_multiplier=1,
                                    pattern=[[-1, KX]])
            mask_by_off[off] = m

        # PE warm-up: keep the array busy (HAM at full clock) while the
        # input DMAs stream in; results are never read.
        warm = ps.tile([128, 128], F32, tag="t", bufs=2, name="warm")
        for _ in range(32):
            nc.tensor.transpose(warm[:], idn[:], idn[:])

        qt_sb = {}

        def emit_qt_group(g):
            tiles = []
            for e in range(ND):
                pq = ps.tile([128, GQ], F32, tag=f"q{e % 2}", bufs=1,
                             name=f"qtp{g}_{e}")
                for d in range(ND):
                    nc.tensor.matmul(pq[:], wt[d][:, 128 * e:128 * (e + 1)],
                                     xtt[d][:, GQ * g:GQ * (g + 1)],
                                     start=(d == 0), stop=(d == ND - 1))
                st = pool.tile([128, GQ], F32R, tag=f"qt{e}", bufs=1,
                               name=f"qt{g}_{e}")
                if e % 2 == 0:
                    nc.vector.tensor_copy(st[:], pq[:])
                else:
                    nc.scalar.copy(st[:], pq[:])
                tiles.append(st)
            qt_sb[g] = tiles

        state = {}

        def emit_head(i):
            g = i // (GQ // QTL)
            qloc = (i % (GQ // QTL)) * QTL
            KX = 256
            ks = min(max(128 * (i - 1), 0), SEQ - KW)          # aligned PV window
            kx = min(max(128 * i - W2, 0), SEQ - KX)           # exact scores window
            off = 128 * i - kx
            pad = kx - ks
            sp = ps.tile([128, KX], F32, tag="s", bufs=2, name=f"s{i}")
            for e in range(ND):
                nc.tensor.matmul(sp[:], qt_sb[g][e][:, qloc:qloc + QTL],
                                 xtt[e][:, kx:kx + KX],
                                 start=(e == 0), stop=(e == ND - 1))
            sm = pool.tile([128, KX], F32, tag="sm", bufs=2, name=f"sm{i}")
            nc.vector.tensor_tensor(out=sm[:], in0=sp[:], in1=mask_by_off[off][:],
                                    op=ALU.add)
            negmax = pool.tile([128, 1], F32, tag="nm", bufs=2, name=f"nm{i}")
            nc.vector.tensor_reduce(negmax[:], sm[:], axis=X, op=ALU.max, negate=True)
            probs = pool.tile([128, KW], F32, tag="pb", bufs=2, name=f"pb{i}")
            nc.gpsimd.memset(probs[:], 0.0)
            sums = pool.tile([128, 1], F32, tag="sums", bufs=2, name=f"sums{i}")
            nc.scalar.activation(probs[:, pad:pad + KX], sm[:], AF.Exp,
                                 bias=negmax[:], scale=1.0, accum_out=sums[:])
            recip = pool.tile([128, 1], F32, tag="recip", bufs=2, name=f"recip{i}")
            nc.vector.reciprocal(recip[:], sums[:])
            state[i] = (probs, recip, ks, pad)

        def emit_tail(i):
            probs, recip, ks, pad = state.pop(i)
            tp = ps.tile([128, KW], F32, tag="t", bufs=2, name=f"tp{i}")
            for j in range(KW // 128):
                nc.tensor.transpose(tp[:, 128 * j:128 * (j + 1)],
                                    probs[:, 128 * j:128 * (j + 1)], idn[:])
            probsT = pool.tile([128, KW], F32R, tag="pt", bufs=2, name=f"pt{i}")
            if i % 2 == 0:
                nc.vector.tensor_copy(probsT[:], tp[:])
            else:
                nc.scalar.copy(probsT[:], tp[:])
            ra = ps.tile([128, HID], F32, tag="ra", bufs=1, name=f"ra{i}")
            kc = ks // 128
            # probs are nonzero only in [pad, pad+256): edge tiles skip one chunk.
            if pad == 0:
                plan = [0, 1]
            elif pad == 128:
                plan = [1, 2]
            else:
                plan = [0, 1, 2]
            for h in range(2):
                cols = slice(512 * h, 512 * (h + 1))
                for n, j in enumerate(plan):
                    nc.tensor.matmul(ra[:, cols],
                                     probsT[:, 128 * j:128 * (j + 1)],
                                     xnt[kc + j][:, cols],
                                     start=(n == 0), stop=(n == len(plan) - 1))
            relu_ra = pool.tile([128, HID], F32, tag="rr", bufs=2, name=f"rr{i}")
            nc.scalar.activation(relu_ra[:], ra[:], AF.Relu, bias=0.0, scale=recip[:])
            ot = pool.tile([128, HID], F32, tag="ot", bufs=2, name=f"ot{i}")
            nc.gpsimd.tensor_tensor(out=ot[:], in0=relu_ra[:],
                                    in1=xnt[i][:].bitcast(F32), op=ALU.add)
            nc.sync.dma_start(out[128 * i:128 * (i + 1), :], ot[:])

        emit_qt_group(0)
        for i in range(NQ + 1):
            if i < NQ:
                if i % 4 == 2 and i // 4 + 1 < NG:
                    emit_qt_group(i // 4 + 1)
                emit_head(i)
            if i >= 1:
                emit_tail(i - 1)

    return nc


def _run(x_all, W, trace=False, tmpdir=None, trace_cores=None):
    import concourse.bass as bass
    from concourse import bass_utils

    nc = bass.Bass("TRN2", target_bir_lowering=False, debug=False, num_devices=8)
    _build(nc)
    _legalize_waits(nc)

    in_maps = []
    for c in range(8):
        in_maps.append({
            "w": W,
            "xt": np.ascontiguousarray(x_all[c].T),
            "xn": np.ascontiguousarray(x_all[c]),
        })
    kwargs = {}
    if trace:
        kwargs = dict(trace=True, tmpdir=tmpdir,
                      trace_cores=trace_cores if trace_cores is not None else [0])
    res = bass_utils.run_bass_kernel_spmd(nc, in_maps, core_ids=list(range(8)),
                                          **kwargs)
    out = np.stack([r["out"] for r in res.results]).astype(np.float32)
    return out, res


def kernel(repr, W):
    x_all = np.ascontiguousarray(np.asarray(repr, dtype=np.float32))
    Wm = np.ascontiguousarray(np.asarray(W, dtype=np.float32))
    out, _ = _run(x_all, Wm, trace=False)
    return out


# Alias for external drivers that expect a `build(nc)` entry point.
build = _build

